# revision 55
# baseline (speedup 1.0000x reference)
"""Fused linear + cross-entropy loss (BaseChunkLoss) on 8 trn2 NeuronCores.

Hybrid sharding (2-way tokens x 4-way vocab, per the hint's tensor-parallel
option): core c = (a, b) with a = c // 4 (token half), b = c % 4 (vocab
quarter) handles 4096 tokens x 8000 vocab. Each core reads only W[:, b-slice]
(65.5 MB) and h[:, a-half] (33.5 MB) -- ~100 MB/core vs 270 MB for pure
token sharding -- so DMA (~344 us modeled) drops below the fp8 PE roofline
(~427 us) and the kernel becomes tensor-engine-bound.

Device layout: tokens on PSUM partitions, vocab on the free dim.
  stationary lhsT = hidden^T tile [128 d x (2 x 128 tok)] fp8
  moving rhs      = weight^T tile [128 d x (2 x 500 vocab)] fp8
Matmuls in fp8e5m2 DoubleRow (K=256/pass, 0.5 cyc/col). e5m2's exponent
range covers W (~0.02) without a pre-scale, so every fp32->fp8 conversion
is a plain tensor_copy: W conversions run on the otherwise-idle Pool
(gpsimd, the only compute op it supports), h conversions split
Pool/DVE/ACT. Per (m-block, 1000-vocab group): DVE adds bias writing bf16
logits to SBUF (freeing the psum bank after one op), then ACT computes exp
there with a fused row-sum accumulator -> s partial per token.

Schedule: phase A streams h (93us) while computing groups 0..2
token-chunk-major with a stagger (group g starts at window g, right after
its W group lands); the phase-B prefetch group's W loads entirely after
the h stream, hidden under phase A's compute tail, so it neither delays
h-dependent tiles nor head-blocks the SP DMA queue. Phase B runs groups
3..7 m-major, one W group prefetched ahead, PE-bound at the fp8 roofline.

Cross-device reduction (the wrapper's all_reduce): host sums the per-core
partial exp-sums over the 4 vocab shards, takes log, and combines with the
exact target logit. The target logit h . W[label] is computed on device as
an fp32 rowdot: each token's label lives in exactly one vocab quarter, so
the host routes (h row, W[label] row) pairs to the owning core, padded to a
fixed 1280 rows (actual counts ~1024 +- 60).

Host-side input prep is layout-only (transpose/slice/gather); all FLOPs
over hidden/weights happen on device inside the measured kernel.
"""
import numpy as np
from contextlib import ExitStack

from concourse import bacc, mybir, tile
from concourse.bass_utils import run_bass_kernel_spmd

F32 = mybir.dt.float32
BF16 = mybir.dt.bfloat16
FP8 = mybir.dt.float8e5
Alu = mybir.AluOpType
Act = mybir.ActivationFunctionType

N_CORES = 8
N_TOK = 8192
D = 2048
V = 32000
P = 128

A_SHARD = 2            # token shards
B_SHARD = 4            # vocab shards
T = N_TOK // A_SHARD   # 4096 tokens per core
VC = V // B_SHARD      # 8000 vocab per core

KP2 = D // 256         # 8 DoubleRow passes of K=256
BANK = 500             # vocab columns per psum bank (<= 512 fp32)
NB = 2                 # banks per vocab group
GV = NB * BANK         # 1000 vocab per group
NG = VC // GV          # 8 groups per core
MB = T // P            # 32 token blocks per core

TPAD = 1280            # padded rowdot rows per core (actual max 1074)
RB = TPAD // P         # 10 rowdot blocks

HQ = 512               # h staging chunk (tokens per DMA)
AG = 3                 # groups computed in the staggered token-chunk phase A
HS_BUFS = 5            # hstage ring depth
WS_BUFS = 3            # wstage ring depth
CONV_SPLIT = False     # phase-A W conversions split DVE/ACT/Pool vs all-Pool
INTERLEAVE_W = False   # interleave W chunks with h chunks inside a window
ET_SBUF = True         # descale writes bf16 logits to SBUF; exp reads there
DS = 2                 # phase-A stagger stride (group g starts at window DS*g)


def _build():
    nc = bacc.Bacc("TRN2", target_bir_lowering=False, debug=False)
    h_d = nc.declare_dram_parameter("h", [D, T], F32, isOutput=False)
    W_d = nc.declare_dram_parameter("W", [D, VC], F32, isOutput=False)
    bias_d = nc.declare_dram_parameter("bias", [VC], F32, isOutput=False)
    hn_d = nc.declare_dram_parameter("hn", [TPAD, D], F32, isOutput=False)
    wg_d = nc.declare_dram_parameter("wg", [TPAD, D], F32, isOutput=False)
    s_out = nc.declare_dram_parameter("s_out", [P, MB], F32, isOutput=True)
    t_out = nc.declare_dram_parameter("t_out", [P, RB], F32, isOutput=True)

    # d = kp*256 + j*128 + ki : row pairs (d, d+128) share a partition, as
    # DoubleRow consumes them from the j free dim.
    W_r2 = W_d[:].rearrange("(kp j ki) v -> kp ki j v", ki=P, j=2)
    h_r2 = h_d[:].rearrange("(kp j ki) t -> kp ki j t", ki=P, j=2)

    with tile.TileContext(nc) as tc, ExitStack() as ctx:
        hpool = ctx.enter_context(tc.tile_pool(name="hT", bufs=1))
        hstage = ctx.enter_context(tc.tile_pool(name="hstage", bufs=HS_BUFS))
        wpool = ctx.enter_context(tc.tile_pool(name="w", bufs=AG + 1))
        wstage = ctx.enter_context(tc.tile_pool(name="wstage", bufs=WS_BUFS))
        bpool = ctx.enter_context(tc.tile_pool(name="bias", bufs=1))
        bstage = ctx.enter_context(tc.tile_pool(name="bstage", bufs=1))
        rpool = ctx.enter_context(tc.tile_pool(name="rowdot", bufs=1))
        epool = (ctx.enter_context(tc.tile_pool(name="et", bufs=3))
                 if ET_SBUF else None)
        djunk = ctx.enter_context(tc.tile_pool(name="djunk", bufs=1))
        pspool = ctx.enter_context(tc.tile_pool(name="ps", bufs=4, space="PSUM"))
        acc = ctx.enter_context(tc.tile_pool(name="acc", bufs=1))

        s_cols = acc.tile([P, MB * NG], F32, tag="scols")
        t_fin = acc.tile([P, RB], F32, tag="tfin")
        t_half = acc.tile([P, 2 * RB], F32, tag="thalf")

        # Pre-warm the Exp table so the 1.3us LoadActFuncSet overlaps the
        # initial DMA lead-in instead of the first psum drain.
        warm = acc.tile([P, 2], F32, tag="warm")
        nc.vector.memset(warm[:], 0.0)
        nc.scalar.activation(warm[:], warm[:], Act.Exp)

        hT = hpool.tile([P, KP2, 2, T], FP8, tag="hT")

        def load_h(kp, tq):
            # h copies split Pool/DVE/ACT so no single engine paces the
            # h-streaming phase.
            st = hstage.tile([P, 2, HQ], F32, tag="hstage")
            nc.sync.dma_start(st[:], h_r2[kp][:, :, tq * HQ:(tq + 1) * HQ])
            dst = hT[:, kp, :, tq * HQ:(tq + 1) * HQ]
            if kp < 3:
                nc.gpsimd.tensor_copy(dst, st[:])
            elif kp < 6:
                nc.vector.tensor_copy(dst, st[:])
            else:
                nc.scalar.copy(dst, st[:])

        wtiles = {}

        def load_w_chunk(g, kp):
            if g not in wtiles:
                wtiles[g] = wpool.tile([P, KP2, 2, GV], FP8, tag="w", name=f"wv{g}")
            ws = wstage.tile([P, 2, GV], F32, tag="wstage")
            nc.sync.dma_start(ws[:], W_r2[kp][:, :, g * GV:(g + 1) * GV])
            nc.gpsimd.tensor_copy(wtiles[g][:, kp], ws[:])

        btiles = {}

        def load_bias(g):
            btiles[g] = bpool.tile([P, GV], BF16, tag=f"bias{g}", name=f"bias{g}")
            bs = bstage.tile([P, GV], F32, tag="bstage")
            nc.sync.dma_start(
                bs[:], bias_d[g * GV:(g + 1) * GV].partition_broadcast(P))
            nc.vector.tensor_copy(btiles[g][:], bs[:])

        DH = D // 2

        def rowdot(r):
            for half in range(2):
                dsl = slice(half * DH, (half + 1) * DH)
                hg = rpool.tile([P, DH], F32, tag="hg")
                nc.sync.dma_start(hg[:], hn_d[r * P:(r + 1) * P, dsl])
                wgt = rpool.tile([P, DH], F32, tag="wgt")
                nc.sync.dma_start(wgt[:], wg_d[r * P:(r + 1) * P, dsl])
                dj = djunk.tile([P, DH], BF16, tag="djunk")
                nc.vector.scalar_tensor_tensor(
                    dj[:], hg[:], 1.0, wgt[:], op0=Alu.mult, op1=Alu.mult,
                    accum_out=t_half[:, 2 * r + half:2 * r + half + 1])

        def drain(g, m, pt):
            psl = pt[:, :, 0:BANK]
            bbv = btiles[g][:].rearrange("p (b c) -> p b c", c=BANK)
            if ET_SBUF:
                # psum bank frees after the DVE pass; exp drains from SBUF
                et = epool.tile([P, NB, BANK], BF16, tag="et")
                nc.vector.tensor_tensor(et[:], psl, bbv, op=Alu.add)
                nc.scalar.activation(
                    et[:], et[:], Act.Exp,
                    accum_out=s_cols[:, m * NG + g:m * NG + g + 1])
            else:
                nc.vector.tensor_tensor(psl, psl, bbv, op=Alu.add)
                nc.scalar.activation(
                    psl, psl, Act.Exp,
                    accum_out=s_cols[:, m * NG + g:m * NG + g + 1])

        def mm(pt, g, m, kp):
            lhsT = hT[:, kp, :, m * P:(m + 1) * P]
            for bk in range(NB):
                nc.tensor.matmul(
                    pt[:, bk, 0:BANK], lhsT,
                    wtiles[g][:, kp, :, bk * BANK:(bk + 1) * BANK],
                    start=(kp == 0), stop=(kp == KP2 - 1),
                    perf_mode=mybir.MatmulPerfMode.DoubleRow,
                )

        def tile_iter(g, m):
            pt = pspool.tile([P, NB, 512], F32, tag="ps")
            for kp in range(KP2):
                mm(pt, g, m, kp)
            drain(g, m, pt)

        def tile_iter_burst(g, ms):
            # Group g's W is still trickling in: issue kp-outer across all
            # m-tiles so each arriving W chunk advances every tile.
            pts = [pspool.tile([P, NB, 512], F32, tag="ps", name=f"psb{m}")
                   for m in ms]
            for kp in range(KP2):
                for pt, m in zip(pts, ms):
                    mm(pt, g, m, kp)
            for pt, m in zip(pts, ms):
                drain(g, m, pt)

        # --- phase A: staggered token-chunk-major. Group g's compute starts
        # at window DS*g, so each W group's 23us of DMA spreads over DS
        # h windows instead of landing whole in one. Window w: h tq w
        # arrives; group g computes m-blocks of tq (w - DS*g).
        NTQ = T // HQ
        MQ = HQ // P           # m-blocks per h chunk
        for w in range(NTQ + AG - 1):
            if w < NTQ:
                for kp in range(KP2):
                    load_h(kp, w)
                if w < AG:
                    for kp in range(KP2):
                        load_w_chunk(w, kp)
                    load_bias(w)
                elif w == NTQ - 1:
                    # phase-B prefetch group AG loads entirely AFTER the h
                    # stream: it is only needed when phase A's compute tail
                    # ends (~20us after the last h window), so it hides under
                    # that tail instead of delaying every h-dependent tile
                    load_bias(AG)
                    for kp in range(KP2):
                        load_w_chunk(AG, kp)
            for g in range(AG):
                tq = w - g
                if tq == 0:
                    tile_iter_burst(g, range(MQ))
                elif 0 < tq < NTQ:
                    for m in range(MQ * tq, MQ * (tq + 1)):
                        tile_iter(g, m)

        # --- phase B: groups AG..NG-1, one W group prefetched ahead ---
        for g in range(AG, NG):
            if g + 1 < NG:
                load_bias(g + 1)
                for kp in range(KP2):
                    load_w_chunk(g + 1, kp)
            r0 = (RB * (g - AG)) // (NG - AG)
            r1 = (RB * (g - AG + 1)) // (NG - AG)
            for r in range(r0, r1):
                rowdot(r)
            for m in range(MB):
                tile_iter(g, m)

        s_fin = acc.tile([P, MB], F32, tag="sfin")
        for m in range(MB):
            nc.vector.tensor_reduce(
                s_fin[:, m:m + 1], s_cols[:, m * NG:(m + 1) * NG],
                axis=mybir.AxisListType.X, op=Alu.add)
        for r in range(RB):
            nc.vector.tensor_tensor(
                t_fin[:, r:r + 1], t_half[:, 2 * r:2 * r + 1],
                t_half[:, 2 * r + 1:2 * r + 2], op=Alu.add)
        nc.sync.dma_start(s_out[:], s_fin[:])
        nc.sync.dma_start(t_out[:], t_fin[:])

    nc.compile()
    return nc


_NC_CACHE = {}


def _get_program():
    if "nc" not in _NC_CACHE:
        _NC_CACHE["nc"] = _build()
    return _NC_CACHE["nc"]


def kernel(hidden_states, head_weight, head_bias, loss_weight, labels,
           chunk_size=None, **_unused):
    hidden = np.asarray(hidden_states, dtype=np.float32)
    W = np.asarray(head_weight, dtype=np.float32)
    bias = np.asarray(head_bias, dtype=np.float32)
    lw = np.asarray(loss_weight, dtype=np.float32)
    labels = np.asarray(labels).astype(np.int64)

    assert hidden.shape == (N_TOK, D) and W.shape == (V, D)

    nc = _get_program()
    Wt = np.ascontiguousarray(W.T)                 # [D, V]
    ht = np.ascontiguousarray(hidden.T)            # [D, N]

    in_maps = []
    core_idx = []                                  # rowdot token indices
    for c in range(N_CORES):
        a, b = c // B_SHARD, c % B_SHARD
        tsl = slice(a * T, (a + 1) * T)
        vlo = b * VC
        lab_c = labels[tsl]
        idx = np.nonzero((lab_c >= vlo) & (lab_c < vlo + VC))[0]
        assert len(idx) <= TPAD, f"core {c}: {len(idx)} rowdot rows > {TPAD}"
        core_idx.append(idx)
        hn = np.zeros((TPAD, D), dtype=np.float32)
        hn[:len(idx)] = hidden[tsl][idx]
        wg = np.zeros((TPAD, D), dtype=np.float32)
        wg[:len(idx)] = W[lab_c[idx]]
        in_maps.append(dict(
            h=np.ascontiguousarray(ht[:, tsl]),
            W=np.ascontiguousarray(Wt[:, vlo:vlo + VC]),
            bias=np.ascontiguousarray(bias[vlo:vlo + VC]),
            hn=hn, wg=wg))

    res = run_bass_kernel_spmd(nc, in_maps, list(range(N_CORES)))

    # unshard + host-side combine (the "all_reduce" of the hint)
    s = np.zeros(N_TOK, dtype=np.float64)
    tgt = np.zeros(N_TOK, dtype=np.float64)
    for c in range(N_CORES):
        a = c // B_SHARD
        tsl = slice(a * T, (a + 1) * T)
        # token t = m*128 + p  ->  s_out[p, m]
        s[tsl] += res.results[c]["s_out"].T.reshape(-1).astype(np.float64)
        idx = core_idx[c]
        td = res.results[c]["t_out"].T.reshape(-1)[:len(idx)]
        tgt[a * T + idx] = td
    tgt = tgt + bias[labels].astype(np.float64)

    lse = np.log(s)
    nll = lse - tgt
    w64 = lw.astype(np.float64)
    loss = (w64 * nll).sum() / max(w64.sum(), 1.0)
    return np.float32(loss)
